# revision 20
# baseline (speedup 1.0000x reference)
"""Trainium2 Bass kernel for nn_MultiHeadAttention_17386027615012.

MHA variant where softmax runs over the HEAD axis (dim=1) and the 1/sqrt(emb)
scale is applied AFTER the softmax. Since softmax is over heads, every (q, k)
position is independent -> shard the flattened (batch, seq) query rows across
the 8 cores (batch b = core // 4, q-chunk of 1024 rows = core % 4) with zero
inter-core communication; each core computes its batch's full K/V.

Math per core (b, q0):
  Q = x[b, q0:q0+1024] @ Wq           (fp32, f32r matmuls)  [1024, 8, 96]
  K = x[b] @ Wk                       (fp32)                [4096, 8, 96]
  V = x[b] @ (Wv / sqrt(768))         (bf16)                [4096, 8, 96]
  e[h,q,k] = Q . K                    (f32r PE, [k,q]-transposed tiles)
  att = exp(e) / sum_h exp(e)         (ACT exp -> bf16, DVE tree sum,
                                       fp32 fast reciprocal)
  O[h,q,:] = sum_k att * V            (bf16 PE matmul, fp32 PSUM accum)
  out = concat_h(O) @ proj_w (+proj_b host-side)

float32r = full-rate fp32 on the PE (plain fp32 streams at half rate).
qkv_w columns are (head, dim, {q,k,v})-interleaved; de-interleaved on host.
K/Q staged through DRAM as [d=96, head, seq] so per-head slices are free-dim
slices (no partition straddling).
"""
import sys

sys.path.insert(0, "/opt/trn_rl_repo")

import numpy as np
from contextlib import ExitStack

import concourse.bass as bass
import concourse.tile as tile
from concourse import bacc, mybir
from concourse import bass_utils

F32 = mybir.dt.float32
F32R = mybir.dt.float32r
BF16 = mybir.dt.bfloat16
EXPF = mybir.ActivationFunctionType.Exp

B, N, E, H, D = 2, 4096, 768, 8, 96
NCORES = 4 * B
QC = N // 4          # 1024 q rows per core (single q pass)
SEG = 4              # k-chunks (of 128) per Ot psum accumulation segment
NKC = N // 128       # 32 k-chunks
NSEG = NKC // SEG    # 8


def r_(ap):
    """View an fp32 AP as float32r (full-rate PE matmul, slightly reduced
    internal precision)."""
    return ap.bitcast(F32R)


def build(use_bias: bool):
    nc = bacc.Bacc("TRN2", debug=False)
    xt = nc.dram_tensor("xt", (E, N), F32R, kind="ExternalInput").ap()
    xtq = nc.dram_tensor("xtq", (E, QC), F32R, kind="ExternalInput").ap()
    wq = nc.dram_tensor("wq", (E, E), F32R, kind="ExternalInput").ap()
    wk = nc.dram_tensor("wk", (E, E), F32R, kind="ExternalInput").ap()
    wv = nc.dram_tensor("wv", (E, E), F32R, kind="ExternalInput").ap()
    pw = nc.dram_tensor("pw", (E, E), F32R, kind="ExternalInput").ap()
    bqk = nc.dram_tensor("bqk", (2, H, D), F32, kind="ExternalInput").ap()
    bv = nc.dram_tensor("bv", (1, E), F32, kind="ExternalInput").ap()
    out = nc.dram_tensor("out", (QC, E), F32, kind="ExternalOutput").ap()

    NE = E // 128  # 6 e-chunks

    with tile.TileContext(nc) as tc, ExitStack() as ctx:
        dram = ctx.enter_context(tc.tile_pool(name="dram", bufs=1, space="DRAM"))
        kt_d = dram.tile([D, H, N], F32R, name="kt_d")     # [d, h, k]
        qt_d = dram.tile([D, H, QC], F32R, name="qt_d")    # [d, h, q]
        v_d = dram.tile([N, E], BF16, name="v_d")         # [k, (h d)]

        # ---------------- Phase A: projections ----------------
        with ExitStack() as actx:
            apool = actx.enter_context(tc.tile_pool(name="apool", bufs=1))
            stg = actx.enter_context(tc.tile_pool(name="stg", bufs=3))
            xts, wqs, wks, wvs = [], [], [], []
            for e in range(NE):
                xt_t = apool.tile([128, N], F32R, name=f"xt{e}")
                nc.sync.dma_start(xt_t[:], xt[e * 128:(e + 1) * 128, :])
                xts.append(xt_t)
                for lst, src, nm in ((wqs, wq, "wq"), (wks, wk, "wk"), (wvs, wv, "wv")):
                    w_t = apool.tile([128, E], F32R, name=f"{nm}{e}")
                    nc.sync.dma_start(w_t[:], src[e * 128:(e + 1) * 128, :])
                    lst.append(w_t)
            if use_bias:
                bqk_t = apool.tile([D, 2 * H], F32, name="bqk_t")
                nc.sync.dma_start(
                    bqk_t.rearrange("d (c h) -> d c h", c=2),
                    bqk.rearrange("c h d -> d c h"),
                )
                ones_t = apool.tile([1, 128], F32, name="ones_t")
                nc.vector.memset(ones_t[:], 1.0)
                bv_t = apool.tile([1, E], F32, name="bv_t")
                nc.sync.dma_start(bv_t[:], bv[:, :])

            # K and Q projections (out [d, k/q] per head -> [d, h, seq] DRAM)
            with tc.tile_pool(name="kqp", bufs=6, space="PSUM") as kqp:
                for h in range(H):
                    for kbq in range(2):  # quads of 512-col blocks
                        kps = []
                        for kb in range(4):
                            kp = kqp.tile([D, 512], F32, name="kp", tag="kp")
                            kps.append(kp)
                        for e in range(NE):
                            for kb in range(4):
                                c0 = (kbq * 4 + kb) * 512
                                nc.tensor.matmul(
                                    kps[kb][:],
                                    wks[e][:, h * D:(h + 1) * D],
                                    xts[e][:, c0:c0 + 512],
                                    start=(e == 0), stop=(e == NE - 1),
                                )
                        for kb in range(4):
                            c0 = (kbq * 4 + kb) * 512
                            kst = stg.tile([D, 512], F32R, name="kst")
                            if use_bias:
                                nc.scalar.activation(
                                    kst[:], kps[kb][:],
                                    mybir.ActivationFunctionType.Identity,
                                    bias=bqk_t[:, H + h:H + h + 1],
                                )
                            else:
                                nc.scalar.copy(kst[:], kps[kb][:])
                            nc.sync.dma_start(kt_d[:, h, c0:c0 + 512], kst[:])
                # Q projection from xtq (streamed per 512-col block)
                for qb in range(2):
                    xtqs = []
                    for e in range(NE):
                        xtq_t = stg.tile([128, 512], F32R, name="xtq_t",
                                         tag="xtq_t", bufs=NE + 2)
                        nc.sync.dma_start(
                            xtq_t[:],
                            xtq[e * 128:(e + 1) * 128, qb * 512:(qb + 1) * 512])
                        xtqs.append(xtq_t)
                    for h in range(H):
                        qp = kqp.tile([D, 512], F32, name="qp", tag="kp")
                        for e in range(NE):
                            nc.tensor.matmul(
                                qp[:],
                                wqs[e][:, h * D:(h + 1) * D],
                                xtqs[e][:],
                                start=(e == 0), stop=(e == NE - 1),
                            )
                        qst = stg.tile([D, 512], F32R, name="qst")
                        if use_bias:
                            nc.scalar.activation(
                                qst[:], qp[:],
                                mybir.ActivationFunctionType.Identity,
                                bias=bqk_t[:, h:h + 1],
                            )
                        else:
                            nc.scalar.copy(qst[:], qp[:])
                        nc.sync.dma_start(
                            qt_d[:, h, qb * 512:(qb + 1) * 512], qst[:]
                        )

            # V projection (out [k, (h d)] bf16 to DRAM, scale pre-folded)
            with tc.tile_pool(name="vp", bufs=3, space="PSUM") as vpp:
                for kc in range(NKC):
                    vp = vpp.tile([128, E], F32, name="vp")
                    for e in range(NE):
                        nc.tensor.matmul(
                            vp[:, 0:512],
                            xts[e][:, kc * 128:(kc + 1) * 128],
                            wvs[e][:, 0:512],
                            start=(e == 0), stop=(e == NE - 1),
                        )
                        nc.tensor.matmul(
                            vp[:, 512:E],
                            xts[e][:, kc * 128:(kc + 1) * 128],
                            wvs[e][:, 512:E],
                            start=(e == 0), stop=(e == NE - 1),
                        )
                    if use_bias:
                        nc.tensor.matmul(
                            vp[:, 0:512], ones_t[:, 0:128], bv_t[:, 0:512],
                            start=False, stop=True, skip_group_check=True,
                        )
                        nc.tensor.matmul(
                            vp[:, 512:E], ones_t[:, 0:128], bv_t[:, 512:E],
                            start=False, stop=True, skip_group_check=True,
                        )
                    vst = stg.tile([128, E], BF16, name="vst")
                    nc.scalar.copy(vst[:], vp[:])
                    nc.sync.dma_start(v_d[kc * 128:(kc + 1) * 128, :], vst[:])

        # ---------------- Phase B: attention (single 1024-q pass) ------------
        bpool = ctx.enter_context(tc.tile_pool(name="bpool", bufs=1))
        qsl = bpool.tile([D, H * QC], F32R, name="qsl")
        nc.sync.dma_start(
            qsl.rearrange("p (h q) -> p h q", h=H), qt_d[:, :, :])
        ot_sb = []
        for h in range(H):
            o_t = bpool.tile([D, QC], F32, name=f"ot{h}")
            ot_sb.append(o_t)

        with ExitStack() as bctx:
            spool = bctx.enter_context(tc.tile_pool(name="spool", bufs=1))
            kvp = bctx.enter_context(tc.tile_pool(name="kvp", bufs=1))
            epsum = bctx.enter_context(
                tc.tile_pool(name="epsum", bufs=2, space="PSUM"))
            otps = bctx.enter_context(
                tc.tile_pool(name="otps", bufs=2, space="PSUM"))

            for seg in range(NSEG):
                exs, vsbs = [], []
                for kc8 in range(SEG):
                    kc = seg * SEG + kc8
                    ks = kvp.tile([D, H * 128], F32R, name="ks", bufs=3)
                    nc.sync.dma_start(
                        ks.rearrange("p (h k) -> p h k", h=H),
                        kt_d[:, :, kc * 128:(kc + 1) * 128],
                    )
                    v_sb = kvp.tile([128, E], BF16, name="v_sb", bufs=SEG + 3)
                    nc.sync.dma_start(v_sb[:], v_d[kc * 128:(kc + 1) * 128, :])
                    vsbs.append(v_sb)

                    kc_exs = []
                    for h in range(H):
                        pe = epsum.tile([128, QC], F32, name="pe")
                        for i in range(2):
                            nc.tensor.matmul(
                                pe[:, i * 512:(i + 1) * 512],
                                ks[:, h * 128:(h + 1) * 128],
                                qsl[:, h * QC + i * 512:h * QC + (i + 1) * 512],
                                start=True, stop=True,
                            )
                        ex = kvp.tile([128, QC], BF16, name="ex",
                                      tag="ex", bufs=H * SEG + 6)
                        nc.scalar.activation(ex[:], pe[:], EXPF)
                        kc_exs.append(ex)
                    # softmax over heads: DVE tree sum + fast reciprocal
                    s01 = spool.tile([128, QC], BF16, name="s01")
                    s23 = spool.tile([128, QC], BF16, name="s23")
                    s45 = spool.tile([128, QC], BF16, name="s45")
                    s67 = spool.tile([128, QC], BF16, name="s67")
                    nc.vector.tensor_add(s01[:], kc_exs[0][:], kc_exs[1][:])
                    nc.vector.tensor_add(s23[:], kc_exs[2][:], kc_exs[3][:])
                    nc.vector.tensor_add(s45[:], kc_exs[4][:], kc_exs[5][:])
                    nc.vector.tensor_add(s67[:], kc_exs[6][:], kc_exs[7][:])
                    s0123 = spool.tile([128, QC], BF16, name="s0123")
                    s4567 = spool.tile([128, QC], BF16, name="s4567")
                    nc.vector.tensor_add(s0123[:], s01[:], s23[:])
                    nc.vector.tensor_add(s4567[:], s45[:], s67[:])
                    sful = spool.tile([128, QC], F32, name="sful", bufs=2)
                    nc.vector.tensor_add(sful[:], s0123[:], s4567[:])
                    r32 = spool.tile([128, QC], F32, name="r32", bufs=2)
                    nc.vector.reciprocal_approx_fast(r32[:], sful[:])
                    r16 = spool.tile([128, QC], BF16, name="r16", bufs=2)
                    nc.vector.tensor_scalar_min(r16[:], r32[:], 3e38)
                    # att = exp * r (out-of-place, all DVE; frees ex slots)
                    kc_atts = []
                    for h in range(H):
                        att = kvp.tile([128, QC], BF16, name="att",
                                       tag="ex", bufs=H * SEG + 6)
                        eng = nc.gpsimd if h >= 5 else nc.vector
                        eng.tensor_mul(att[:], kc_exs[h][:], r16[:])
                        kc_atts.append(att)
                    exs.append(kc_atts)

                # O[h] += att[h].T-free matmul over this segment's k-chunks
                for h in range(H):
                    otp = otps.tile([D, QC], F32, name="otp")
                    for kc8 in range(SEG):
                        for i in range(2):
                            nc.tensor.matmul(
                                otp[:, i * 512:(i + 1) * 512],
                                vsbs[kc8][:, h * D:(h + 1) * D],
                                exs[kc8][h][:, i * 512:(i + 1) * 512],
                                start=(kc8 == 0), stop=(kc8 == SEG - 1),
                            )
                    if seg == 0:
                        nc.scalar.copy(ot_sb[h][:], otp[:])
                    else:
                        nc.vector.tensor_add(ot_sb[h][:], ot_sb[h][:], otp[:])

        # ---------------- Phase C: output projection ----------------
        pwp = ctx.enter_context(tc.tile_pool(name="pwp", bufs=1))
        pws = []
        for h in range(H):
            pw_t = pwp.tile([D, E], F32R, name=f"pw{h}")
            nc.sync.dma_start(pw_t[:], pw[h * D:(h + 1) * D, :])
            pws.append(pw_t)
        ostp = ctx.enter_context(tc.tile_pool(name="ostp", bufs=2))
        otr = []
        for h in range(H):
            otr_t = pwp.tile([D, QC], F32R, name=f"otr{h}")
            nc.scalar.copy(otr_t[:], ot_sb[h][:])
            otr.append(otr_t)
        with tc.tile_pool(name="pop", bufs=2, space="PSUM") as pop:
            for qb in range(QC // 128):
                po = pop.tile([128, E], F32, name="po")
                for h in range(H):
                    lhs = otr[h][:, qb * 128:(qb + 1) * 128]
                    nc.tensor.matmul(
                        po[:, 0:512], lhs, pws[h][:, 0:512],
                        start=(h == 0), stop=(h == H - 1))
                    nc.tensor.matmul(
                        po[:, 512:E], lhs, pws[h][:, 512:E],
                        start=(h == 0), stop=(h == H - 1))
                ost = ostp.tile([128, E], F32, name="ost")
                nc.scalar.copy(ost[:], po[:])
                nc.sync.dma_start(out[qb * 128:(qb + 1) * 128, :], ost[:])

    nc.compile()
    return nc


_CACHE = {}


def _get_program(use_bias: bool):
    if use_bias not in _CACHE:
        _CACHE[use_bias] = build(use_bias)
    return _CACHE[use_bias]


def _prep_inputs(x, qkv_w, qkv_b, proj_w):
    qw = np.ascontiguousarray(qkv_w.reshape(E, H, D, 3))
    wq = np.ascontiguousarray(qw[..., 0].reshape(E, E))
    wk = np.ascontiguousarray(qw[..., 1].reshape(E, E))
    wv = np.ascontiguousarray(qw[..., 2].reshape(E, E)) / np.sqrt(np.float32(E))
    qb = qkv_b.reshape(H, D, 3)
    bqk = np.ascontiguousarray(
        np.stack([qb[..., 0], qb[..., 1]], axis=0)).astype(np.float32)
    bv = np.ascontiguousarray(
        qb[..., 2].reshape(1, E)).astype(np.float32) / np.sqrt(np.float32(E))
    xts = [np.ascontiguousarray(x[b].T) for b in range(B)]
    in_maps = []
    for c in range(NCORES):
        b, qi = c // 4, c % 4
        in_maps.append({
            "xt": xts[b],
            "xtq": np.ascontiguousarray(xts[b][:, qi * QC:(qi + 1) * QC]),
            "wq": wq, "wk": wk, "wv": wv.astype(np.float32),
            "pw": np.ascontiguousarray(proj_w.astype(np.float32)),
            "bqk": bqk, "bv": bv,
        })
    return in_maps


def kernel(x, qkv_w, qkv_b, proj_w, proj_b, _trace=False):
    x = np.asarray(x, dtype=np.float32)
    qkv_w = np.asarray(qkv_w, dtype=np.float32)
    qkv_b = np.asarray(qkv_b, dtype=np.float32)
    proj_w = np.asarray(proj_w, dtype=np.float32)
    proj_b = np.asarray(proj_b, dtype=np.float32)

    use_bias = bool(np.any(qkv_b))
    nc = _get_program(use_bias)
    in_maps = _prep_inputs(x, qkv_w, qkv_b, proj_w)
    res = bass_utils.run_bass_kernel_spmd(
        nc, in_maps, core_ids=list(range(NCORES)), trace=_trace)
    outf = np.empty((B, N, E), dtype=np.float32)
    for c in range(NCORES):
        b, qi = c // 4, c % 4
        outf[b, qi * QC:(qi + 1) * QC, :] = res.results[c]["out"]
    if np.any(proj_b):
        outf += proj_b[None, None, :]
    if _trace:
        kernel.last_exec_time_ns = res.exec_time_ns
        kernel.last_results = res
    return outf


# revision 21
# speedup vs baseline: 1.0954x; 1.0954x over previous
"""Trainium2 Bass kernel for nn_MultiHeadAttention_17386027615012.

MHA variant where softmax runs over the HEAD axis (dim=1) and the 1/sqrt(emb)
scale is applied AFTER the softmax. Since softmax is over heads, every (q, k)
position is independent -> shard the flattened (batch, seq) query rows across
the 8 cores (batch b = core // 4, q-chunk of 1024 rows = core % 4) with zero
inter-core communication; each core computes its batch's full K/V.

Math per core (b, q0):
  Q = x[b, q0:q0+1024] @ Wq           (fp32, f32r matmuls)  [1024, 8, 96]
  K = x[b] @ Wk                       (fp32)                [4096, 8, 96]
  V = x[b] @ (Wv / sqrt(768))         (bf16)                [4096, 8, 96]
  e[h,q,k] = Q . K                    (f32r PE, [k,q]-transposed tiles)
  att = exp(e) / sum_h exp(e)         (ACT exp -> bf16, DVE tree sum,
                                       fp32 fast reciprocal)
  O[h,q,:] = sum_k att * V            (bf16 PE matmul, fp32 PSUM accum)
  out = concat_h(O) @ proj_w (+proj_b host-side)

float32r = full-rate fp32 on the PE (plain fp32 streams at half rate).
qkv_w columns are (head, dim, {q,k,v})-interleaved; de-interleaved on host.
K/Q staged through DRAM as [d=96, head, seq] so per-head slices are free-dim
slices (no partition straddling).
"""
import sys

sys.path.insert(0, "/opt/trn_rl_repo")

import numpy as np
from contextlib import ExitStack

import concourse.bass as bass
import concourse.tile as tile
from concourse import bacc, mybir
from concourse import bass_utils

F32 = mybir.dt.float32
F32R = mybir.dt.float32r
BF16 = mybir.dt.bfloat16
EXPF = mybir.ActivationFunctionType.Exp

B, N, E, H, D = 2, 4096, 768, 8, 96
NCORES = 4 * B
QC = N // 4          # 1024 q rows per core (single q pass)
SEG = 4              # k-chunks (of 128) per Ot psum accumulation segment
NKC = N // 128       # 32 k-chunks
NSEG = NKC // SEG    # 8


def r_(ap):
    """View an fp32 AP as float32r (full-rate PE matmul, slightly reduced
    internal precision)."""
    return ap.bitcast(F32R)


def build(use_bias: bool):
    nc = bacc.Bacc("TRN2", debug=False)
    xt = nc.dram_tensor("xt", (E, N), F32R, kind="ExternalInput").ap()
    xtq = nc.dram_tensor("xtq", (E, QC), F32R, kind="ExternalInput").ap()
    wq = nc.dram_tensor("wq", (E, E), F32R, kind="ExternalInput").ap()
    wk = nc.dram_tensor("wk", (E, E), F32R, kind="ExternalInput").ap()
    wv = nc.dram_tensor("wv", (E, E), F32R, kind="ExternalInput").ap()
    pw = nc.dram_tensor("pw", (E, E), F32R, kind="ExternalInput").ap()
    bqk = nc.dram_tensor("bqk", (2, H, D), F32, kind="ExternalInput").ap()
    bv = nc.dram_tensor("bv", (1, E), F32, kind="ExternalInput").ap()
    out = nc.dram_tensor("out", (QC, E), F32, kind="ExternalOutput").ap()

    NE = E // 128  # 6 e-chunks

    with tile.TileContext(nc) as tc, ExitStack() as ctx:
        dram = ctx.enter_context(tc.tile_pool(name="dram", bufs=1, space="DRAM"))
        kt_d = dram.tile([D, H, N], F32R, name="kt_d")     # [d, h, k]
        qt_d = dram.tile([D, H, QC], F32R, name="qt_d")    # [d, h, q]
        v_d = dram.tile([N, E], BF16, name="v_d")         # [k, (h d)]

        # ---------------- Phase A: projections ----------------
        with ExitStack() as actx:
            apool = actx.enter_context(tc.tile_pool(name="apool", bufs=1))
            stg = actx.enter_context(tc.tile_pool(name="stg", bufs=3))
            xts, wqs, wks, wvs = [], [], [], []
            for e in range(NE):
                xt_t = apool.tile([128, N], F32R, name=f"xt{e}")
                nc.sync.dma_start(xt_t[:], xt[e * 128:(e + 1) * 128, :])
                xts.append(xt_t)
                for lst, src, nm in ((wqs, wq, "wq"), (wks, wk, "wk"), (wvs, wv, "wv")):
                    w_t = apool.tile([128, E], F32R, name=f"{nm}{e}")
                    nc.sync.dma_start(w_t[:], src[e * 128:(e + 1) * 128, :])
                    lst.append(w_t)
            if use_bias:
                bqk_t = apool.tile([D, 2 * H], F32, name="bqk_t")
                nc.sync.dma_start(
                    bqk_t.rearrange("d (c h) -> d c h", c=2),
                    bqk.rearrange("c h d -> d c h"),
                )
                ones_t = apool.tile([1, 128], F32, name="ones_t")
                nc.vector.memset(ones_t[:], 1.0)
                bv_t = apool.tile([1, E], F32, name="bv_t")
                nc.sync.dma_start(bv_t[:], bv[:, :])

            # K and Q projections (out [d, k/q] per head -> [d, h, seq] DRAM)
            with tc.tile_pool(name="kqp", bufs=6, space="PSUM") as kqp:
                for h in range(H):
                    for kbq in range(2):  # quads of 512-col blocks
                        kps = []
                        for kb in range(4):
                            kp = kqp.tile([D, 512], F32, name="kp", tag="kp")
                            kps.append(kp)
                        for e in range(NE):
                            for kb in range(4):
                                c0 = (kbq * 4 + kb) * 512
                                nc.tensor.matmul(
                                    kps[kb][:],
                                    wks[e][:, h * D:(h + 1) * D],
                                    xts[e][:, c0:c0 + 512],
                                    start=(e == 0), stop=(e == NE - 1),
                                )
                        for kb in range(4):
                            c0 = (kbq * 4 + kb) * 512
                            kst = stg.tile([D, 512], F32R, name="kst")
                            if use_bias:
                                nc.scalar.activation(
                                    kst[:], kps[kb][:],
                                    mybir.ActivationFunctionType.Identity,
                                    bias=bqk_t[:, H + h:H + h + 1],
                                )
                            else:
                                nc.scalar.copy(kst[:], kps[kb][:])
                            nc.sync.dma_start(kt_d[:, h, c0:c0 + 512], kst[:])
                # Q projection from xtq (streamed per 512-col block)
                for qb in range(2):
                    xtqs = []
                    for e in range(NE):
                        xtq_t = stg.tile([128, 512], F32R, name="xtq_t",
                                         tag="xtq_t", bufs=NE + 2)
                        nc.sync.dma_start(
                            xtq_t[:],
                            xtq[e * 128:(e + 1) * 128, qb * 512:(qb + 1) * 512])
                        xtqs.append(xtq_t)
                    for h in range(H):
                        qp = kqp.tile([D, 512], F32, name="qp", tag="kp")
                        for e in range(NE):
                            nc.tensor.matmul(
                                qp[:],
                                wqs[e][:, h * D:(h + 1) * D],
                                xtqs[e][:],
                                start=(e == 0), stop=(e == NE - 1),
                            )
                        qst = stg.tile([D, 512], F32R, name="qst")
                        if use_bias:
                            nc.scalar.activation(
                                qst[:], qp[:],
                                mybir.ActivationFunctionType.Identity,
                                bias=bqk_t[:, h:h + 1],
                            )
                        else:
                            nc.scalar.copy(qst[:], qp[:])
                        nc.sync.dma_start(
                            qt_d[:, h, qb * 512:(qb + 1) * 512], qst[:]
                        )

            # V projection (out [k, (h d)] bf16 to DRAM, scale pre-folded)
            with tc.tile_pool(name="vp", bufs=3, space="PSUM") as vpp:
                for kc in range(NKC):
                    vp = vpp.tile([128, E], F32, name="vp")
                    for e in range(NE):
                        nc.tensor.matmul(
                            vp[:, 0:512],
                            xts[e][:, kc * 128:(kc + 1) * 128],
                            wvs[e][:, 0:512],
                            start=(e == 0), stop=(e == NE - 1),
                        )
                        nc.tensor.matmul(
                            vp[:, 512:E],
                            xts[e][:, kc * 128:(kc + 1) * 128],
                            wvs[e][:, 512:E],
                            start=(e == 0), stop=(e == NE - 1),
                        )
                    if use_bias:
                        nc.tensor.matmul(
                            vp[:, 0:512], ones_t[:, 0:128], bv_t[:, 0:512],
                            start=False, stop=True, skip_group_check=True,
                        )
                        nc.tensor.matmul(
                            vp[:, 512:E], ones_t[:, 0:128], bv_t[:, 512:E],
                            start=False, stop=True, skip_group_check=True,
                        )
                    vst = stg.tile([128, E], BF16, name="vst")
                    nc.scalar.copy(vst[:], vp[:])
                    nc.sync.dma_start(v_d[kc * 128:(kc + 1) * 128, :], vst[:])

        # ---------------- Phase B: attention (single 1024-q pass) ------------
        bpool = ctx.enter_context(tc.tile_pool(name="bpool", bufs=1))
        qsl = bpool.tile([D, H * QC], F32R, name="qsl")
        nc.sync.dma_start(
            qsl.rearrange("p (h q) -> p h q", h=H), qt_d[:, :, :])
        ot_sb = []
        for h in range(H):
            o_t = bpool.tile([D, QC], F32, name=f"ot{h}")
            ot_sb.append(o_t)

        with ExitStack() as bctx:
            spool = bctx.enter_context(tc.tile_pool(name="spool", bufs=1))
            kvp = bctx.enter_context(tc.tile_pool(name="kvp", bufs=1))
            epsum = bctx.enter_context(
                tc.tile_pool(name="epsum", bufs=2, space="PSUM"))
            otps = bctx.enter_context(
                tc.tile_pool(name="otps", bufs=2, space="PSUM"))

            for seg in range(NSEG):
                exs, vsbs = [], []
                for kc8 in range(SEG):
                    kc = seg * SEG + kc8
                    ks = kvp.tile([D, H * 128], F32R, name="ks", bufs=3)
                    nc.sync.dma_start(
                        ks.rearrange("p (h k) -> p h k", h=H),
                        kt_d[:, :, kc * 128:(kc + 1) * 128],
                    )
                    v_sb = kvp.tile([128, E], BF16, name="v_sb", bufs=SEG + 3)
                    nc.sync.dma_start(v_sb[:], v_d[kc * 128:(kc + 1) * 128, :])
                    vsbs.append(v_sb)

                    kc_exs = []
                    for h in range(H):
                        pe = epsum.tile([128, QC], F32, name="pe")
                        for i in range(2):
                            nc.tensor.matmul(
                                pe[:, i * 512:(i + 1) * 512],
                                ks[:, h * 128:(h + 1) * 128],
                                qsl[:, h * QC + i * 512:h * QC + (i + 1) * 512],
                                start=True, stop=True,
                            )
                        ex = kvp.tile([128, QC], BF16, name="ex",
                                      tag="ex", bufs=H * SEG + 6)
                        nc.scalar.activation(ex[:], pe[:], EXPF)
                        kc_exs.append(ex)
                    # softmax over heads: DVE tree sum + fast reciprocal
                    s01 = spool.tile([128, QC], BF16, name="s01")
                    s23 = spool.tile([128, QC], BF16, name="s23")
                    s45 = spool.tile([128, QC], BF16, name="s45")
                    s67 = spool.tile([128, QC], BF16, name="s67")
                    nc.vector.tensor_add(s01[:], kc_exs[0][:], kc_exs[1][:])
                    nc.vector.tensor_add(s23[:], kc_exs[2][:], kc_exs[3][:])
                    nc.vector.tensor_add(s45[:], kc_exs[4][:], kc_exs[5][:])
                    nc.vector.tensor_add(s67[:], kc_exs[6][:], kc_exs[7][:])
                    s0123 = spool.tile([128, QC], BF16, name="s0123")
                    s4567 = spool.tile([128, QC], BF16, name="s4567")
                    nc.vector.tensor_add(s0123[:], s01[:], s23[:])
                    nc.vector.tensor_add(s4567[:], s45[:], s67[:])
                    sful = spool.tile([128, QC], F32, name="sful", bufs=2)
                    nc.vector.tensor_add(sful[:], s0123[:], s4567[:])
                    r32 = spool.tile([128, QC], F32, name="r32", bufs=2)
                    nc.vector.reciprocal_approx_fast(r32[:], sful[:])
                    r16 = spool.tile([128, QC], BF16, name="r16", bufs=2)
                    nc.vector.tensor_scalar_min(r16[:], r32[:], 3e38)
                    # att = exp * r (out-of-place, all DVE; frees ex slots)
                    kc_atts = []
                    for h in range(H):
                        att = kvp.tile([128, QC], BF16, name="att",
                                       tag="ex", bufs=H * SEG + 6)
                        nc.vector.tensor_mul(att[:], kc_exs[h][:], r16[:])
                        kc_atts.append(att)
                    exs.append(kc_atts)

                # O[h] += att[h].T-free matmul over this segment's k-chunks
                for h in range(H):
                    otp = otps.tile([D, QC], F32, name="otp")
                    for kc8 in range(SEG):
                        for i in range(2):
                            nc.tensor.matmul(
                                otp[:, i * 512:(i + 1) * 512],
                                vsbs[kc8][:, h * D:(h + 1) * D],
                                exs[kc8][h][:, i * 512:(i + 1) * 512],
                                start=(kc8 == 0), stop=(kc8 == SEG - 1),
                            )
                    if seg == 0:
                        nc.scalar.copy(ot_sb[h][:], otp[:])
                    else:
                        nc.vector.tensor_add(ot_sb[h][:], ot_sb[h][:], otp[:])

        # ---------------- Phase C: output projection ----------------
        pwp = ctx.enter_context(tc.tile_pool(name="pwp", bufs=1))
        pws = []
        for h in range(H):
            pw_t = pwp.tile([D, E], F32R, name=f"pw{h}")
            nc.sync.dma_start(pw_t[:], pw[h * D:(h + 1) * D, :])
            pws.append(pw_t)
        ostp = ctx.enter_context(tc.tile_pool(name="ostp", bufs=2))
        otr = []
        for h in range(H):
            otr_t = pwp.tile([D, QC], F32R, name=f"otr{h}")
            nc.scalar.copy(otr_t[:], ot_sb[h][:])
            otr.append(otr_t)
        with tc.tile_pool(name="pop", bufs=2, space="PSUM") as pop:
            for qb in range(QC // 128):
                po = pop.tile([128, E], F32, name="po")
                for h in range(H):
                    lhs = otr[h][:, qb * 128:(qb + 1) * 128]
                    nc.tensor.matmul(
                        po[:, 0:512], lhs, pws[h][:, 0:512],
                        start=(h == 0), stop=(h == H - 1))
                    nc.tensor.matmul(
                        po[:, 512:E], lhs, pws[h][:, 512:E],
                        start=(h == 0), stop=(h == H - 1))
                ost = ostp.tile([128, E], F32, name="ost")
                nc.scalar.copy(ost[:], po[:])
                nc.sync.dma_start(out[qb * 128:(qb + 1) * 128, :], ost[:])

    nc.compile()
    return nc


_CACHE = {}


def _get_program(use_bias: bool):
    if use_bias not in _CACHE:
        _CACHE[use_bias] = build(use_bias)
    return _CACHE[use_bias]


def _prep_inputs(x, qkv_w, qkv_b, proj_w):
    qw = np.ascontiguousarray(qkv_w.reshape(E, H, D, 3))
    wq = np.ascontiguousarray(qw[..., 0].reshape(E, E))
    wk = np.ascontiguousarray(qw[..., 1].reshape(E, E))
    wv = np.ascontiguousarray(qw[..., 2].reshape(E, E)) / np.sqrt(np.float32(E))
    qb = qkv_b.reshape(H, D, 3)
    bqk = np.ascontiguousarray(
        np.stack([qb[..., 0], qb[..., 1]], axis=0)).astype(np.float32)
    bv = np.ascontiguousarray(
        qb[..., 2].reshape(1, E)).astype(np.float32) / np.sqrt(np.float32(E))
    xts = [np.ascontiguousarray(x[b].T) for b in range(B)]
    in_maps = []
    for c in range(NCORES):
        b, qi = c // 4, c % 4
        in_maps.append({
            "xt": xts[b],
            "xtq": np.ascontiguousarray(xts[b][:, qi * QC:(qi + 1) * QC]),
            "wq": wq, "wk": wk, "wv": wv.astype(np.float32),
            "pw": np.ascontiguousarray(proj_w.astype(np.float32)),
            "bqk": bqk, "bv": bv,
        })
    return in_maps


def kernel(x, qkv_w, qkv_b, proj_w, proj_b, _trace=False):
    x = np.asarray(x, dtype=np.float32)
    qkv_w = np.asarray(qkv_w, dtype=np.float32)
    qkv_b = np.asarray(qkv_b, dtype=np.float32)
    proj_w = np.asarray(proj_w, dtype=np.float32)
    proj_b = np.asarray(proj_b, dtype=np.float32)

    use_bias = bool(np.any(qkv_b))
    nc = _get_program(use_bias)
    in_maps = _prep_inputs(x, qkv_w, qkv_b, proj_w)
    res = bass_utils.run_bass_kernel_spmd(
        nc, in_maps, core_ids=list(range(NCORES)), trace=_trace)
    outf = np.empty((B, N, E), dtype=np.float32)
    for c in range(NCORES):
        b, qi = c // 4, c % 4
        outf[b, qi * QC:(qi + 1) * QC, :] = res.results[c]["out"]
    if np.any(proj_b):
        outf += proj_b[None, None, :]
    if _trace:
        kernel.last_exec_time_ns = res.exec_time_ns
        kernel.last_results = res
    return outf
